# revision 53
# baseline (speedup 1.0000x reference)
"""PhaGruMPN3 message-passing GNN on 8 TRN2 NeuronCores (Bass/Tile).

Graph/data-parallel sharding (per the sharding hint): atoms are sharded
contiguously across the 8 cores; the per-pair message table is
partitioned per device in consumption order (halo duplication on the
host, which owns all static index gathers), so each core streams its
pair rows sequentially. W_h is folded into the GRU input weights
(associativity), so the 4M-row `em` table is never materialized.

Device-side structure (launch A): the edge relu-matmul runs in a
4-atom-packed transposed layout ([128 partitions = 4 atom blocks x 32
hidden] x [512 columns = 32 atoms x 16 neighbor slots]), all in fp16
(weights in fp16 keep the end-to-end error ~1.7e-3; every DVE op
qualifies for the 2-byte fast path). The neighbor-slot reduction is
split across engines to balance load: PSUM eviction+relu runs on the
Activation engine (1024-wide pairs amortize its fixed access latency)
and on DVE (512-wide singles) - the Pool engine cannot read PSUM - then
one fp16 halving round on DVE (16->8 slots) and two on Pool (8->2)
leave 2 partials per atom whose final sum rides the GRU gate matmuls
as extra PSUM-accumulated terms (the tensor engine has slack). The GRU
is emitted in two software-pipelined phases trailing stage 1 by two
tiles so no engine convoys on the rh-dependent candidate matmul.
Launches B (one per remaining GRU depth) are plain GRU updates on
1024-wide fp16 tiles (512-wide edge tiles shorten pipeline fill/drain,
and the final tile is trimmed to the real atom count) with front-loaded
DMA and a deep PSUM ring. Stage-1 work for all-padding blocks is
skipped outright (their aggregation is exactly zero), the last GRU tile
of both launches is trimmed to the columns holding real atoms, preamble
weight DMAs issue from the Activation/Pool queues so the first edge
chunk is not serialized behind them on SP, and the last two tiles' GRU
tails are interleaved to hide the serial DVE chain during drain. Between launches
the host applies the composed static-index gather-sum
(b_scope o scope_update) and re-shards.
"""

import sys

sys.path.insert(0, "/opt/trn_rl_repo")

import numpy as np

HID = 32
FEAT = 8
NCORES = 8


def _cfg(n_atoms, depth):
    assert n_atoms % NCORES == 0
    shard = n_atoms // NCORES
    shard_pad = -(-shard // 2048) * 2048
    cols = shard_pad // 4
    return dict(n_atoms=n_atoms, depth=depth, shard=shard, shard_pad=shard_pad,
                cols=cols, nt_gru=cols // 512, nm1=shard_pad // 128)


_NC_CACHE = {}


def _build(kind, cfg):
    """kind 'A': stage1 + h0 + GRU(d=0) -> h1. kind 'B': GRU(one depth)."""
    key = (kind, tuple(sorted(cfg.items())))
    if key in _NC_CACHE:
        return _NC_CACHE[key]
    import concourse.bacc as bacc
    import concourse.tile as tile
    from concourse import mybir

    dt = mybir.dt
    AX = mybir.AxisListType
    OP = mybir.AluOpType
    ACT = mybir.ActivationFunctionType

    COLS = cfg["cols"]
    NT = cfg["nt_gru"]
    NM1 = cfg["nm1"]
    BPT = NM1 // NT        # stage-1 blocks per GRU tile (16)

    nc = bacc.Bacc("TRN2", target_bir_lowering=False, debug=False,
                   enable_asserts=False, num_devices=NCORES)

    if kind == "A":
        xt4 = nc.dram_tensor("xt4", [NM1, 36, 512], dt.float16,
                             kind="ExternalInput")
        tft4 = nc.dram_tensor("tft4", [32, COLS], dt.float16,
                              kind="ExternalInput")
        wia4 = nc.dram_tensor("wia4", [32, 128], dt.float16,
                              kind="ExternalInput")
        wib4 = nc.dram_tensor("wib4", [36, 128], dt.float16,
                              kind="ExternalInput")
    else:
        aggi = nc.dram_tensor("aggi", [128, COLS], dt.float16,
                              kind="ExternalInput")
        hi = nc.dram_tensor("hi", [128, COLS], dt.float16,
                            kind="ExternalInput")
    gruw = nc.dram_tensor("gruw", [6 * 128, 128], dt.float16,
                          kind="ExternalInput")
    biasw = nc.dram_tensor("biasw", [128, 3], dt.float32, kind="ExternalInput")
    out_h = nc.dram_tensor("out_h", [128, COLS], dt.float16,
                           kind="ExternalOutput")

    with tile.TileContext(nc) as tc, \
         tc.tile_pool(name="persist", bufs=1) as pp, \
         tc.tile_pool(name="ps1", bufs=3, space="PSUM") as ps1, \
         tc.tile_pool(name="psg", bufs=2, space="PSUM") as psg, \
         tc.tile_pool(name="sb", bufs=2) as sbp, \
         tc.tile_pool(name="sb3", bufs=4) as sbp3, \
         tc.tile_pool(name="agp", bufs=4) as agp, \
         nc.allow_low_precision(reason="bf16 gnn pipeline"):

        gw = pp.tile([128, 6 * 128], dt.float16, name="gw")

        def emit_gw_dma(eng):
            eng.dma_start(out=gw[:].rearrange("p (i n) -> p i n", n=128),
                          in_=gruw[:].rearrange("(i p) n -> p i n", p=128))

        bw = pp.tile([128, 3], dt.float32, name="bw")
        if kind == "B":
            emit_gw_dma(nc.sync)
            nc.sync.dma_start(out=bw[:], in_=biasw[:])
        else:
            nc.gpsimd.dma_start(out=bw[:], in_=biasw[:])
        hT = pp.tile([128, COLS], dt.float16, name="hT")

        def gw_s(i):
            return gw[:, i * 128:(i + 1) * 128]

        if kind == "A":
            wib = pp.tile([36, 128], dt.float16, name="wib")
            nc.sync.dma_start(out=wib[:], in_=wib4[:])
            wia = pp.tile([32, 128], dt.float16, name="wia")
        else:
            agf = pp.tile([128, COLS], dt.float16, name="agf")

        CREAL = -(-cfg["shard"] // 4)    # cols holding real atoms

        def gtw(t):
            # GRU width of tile t, trimmed to real columns on the last tile
            if kind != "A":
                return 512
            return max(8, min(512, -(-(CREAL - 512 * t) // 8) * 8))

        def gate(cs, wi, rhs2, agg4, w=512):
            pm = psg.tile([128, w], dt.float32, space="PSUM", tag="g",
                          bufs=2)
            if kind == "A":
                for s in range(2):
                    nc.tensor.matmul(pm[:], lhsT=gw_s(2 * wi),
                                     rhs=agg4[:, :w, s],
                                     start=(s == 0), stop=False)
            else:
                nc.tensor.matmul(pm[:], lhsT=gw_s(2 * wi),
                                 rhs=agf[:, cs], start=True, stop=False)
            nc.tensor.matmul(pm[:], lhsT=gw_s(2 * wi + 1), rhs=rhs2,
                             start=False, stop=True)
            return pm

        def emit_gru_zr(t, agg4=None):
            """GRU phase 1 on tile t: gates z, r and rh = r*h."""
            w = gtw(t)
            cs = slice(512 * t, 512 * t + w)
            pz = gate(cs, 0, hT[:, cs], agg4, w)
            z = sbp.tile([128, w], dt.float16, tag="z")
            nc.scalar.activation(z[:], pz[:], ACT.Sigmoid, bias=bw[:, 0:1])
            pr = gate(cs, 1, hT[:, cs], agg4, w)
            r = sbp.tile([128, w], dt.float16, tag="r")
            nc.scalar.activation(r[:], pr[:], ACT.Sigmoid, bias=bw[:, 1:2])
            rh = sbp.tile([128, w], dt.float16, tag="rh")
            nc.vector.tensor_tensor(out=rh[:], in0=r[:], in1=hT[:, cs],
                                    op=OP.mult)
            return z, rh

        def emit_gru_h_pair(ts_):
            """Interleave the final tiles' phase-2 chains to hide the
            serial DVE tail during drain."""
            phs, hcs, ds, zds = [], [], [], []
            for t, z, rh, agg4 in ts_:
                w = gtw(t)
                cs = slice(512 * t, 512 * t + w)
                phs.append((cs, w, gate(cs, 2, rh[:], agg4, w)))
            for cs, w, ph in phs:
                hc = sbp.tile([128, w], dt.float16, tag="hc")
                nc.scalar.activation(hc[:], ph[:], ACT.Tanh, bias=bw[:, 2:3])
                hcs.append((cs, w, hc))
            for cs, w, hc in hcs:
                d = sbp.tile([128, w], dt.float16, tag="d")
                nc.vector.tensor_tensor(out=d[:], in0=hc[:], in1=hT[:, cs],
                                        op=OP.subtract)
                ds.append((cs, w, d))
            for (t, z, rh, agg4), (cs, w, d) in zip(ts_, ds):
                zd = sbp.tile([128, w], dt.float16, tag="zd")
                nc.vector.tensor_tensor(out=zd[:], in0=z[:], in1=d[:],
                                        op=OP.mult)
                zds.append((cs, zd))
            for cs, zd in zds:
                nc.vector.tensor_tensor(out=hT[:, cs], in0=hT[:, cs],
                                        in1=zd[:], op=OP.add)
                nc.sync.dma_start(out=out_h[:, cs], in_=hT[:, cs])

        def emit_gru_h(t, z, rh, agg4=None, tteng=None):
            """GRU phase 2 on tile t: candidate hc and the h update."""
            w = gtw(t)
            te = tteng or nc.vector
            cs = slice(512 * t, 512 * t + w)
            ph = gate(cs, 2, rh[:], agg4, w)
            hc = sbp.tile([128, w], dt.float16, tag="hc")
            nc.scalar.activation(hc[:], ph[:], ACT.Tanh, bias=bw[:, 2:3])
            d = sbp.tile([128, w], dt.float16, tag="d")
            nc.vector.tensor_tensor(out=d[:], in0=hc[:], in1=hT[:, cs],
                                    op=OP.subtract)
            zd = sbp.tile([128, w], dt.float16, tag="zd")
            te.tensor_tensor(out=zd[:], in0=z[:], in1=d[:],
                             op=OP.mult)
            te.tensor_tensor(out=hT[:, cs], in0=hT[:, cs], in1=zd[:],
                             op=OP.add)
            nc.sync.dma_start(out=out_h[:, cs], in_=hT[:, cs])

        if kind == "A":
            import os
            TUNE = dict(n_x=48, n_q8p=0, sbb=4, poff=2)
            for kv in os.environ.get("KTUNE", "").split(","):
                if ":" in kv:
                    k, v = kv.split(":")
                    TUNE[k] = int(v)
            NGRP = NM1 // 4
            n_x = (NGRP * TUNE["n_x"]) // 64
            n_q8p = (NGRP * TUNE["n_q8p"]) // 64
            POFF = TUNE["poff"]
            aggs = {}       # t -> agg4 tile (stage 1 emitted)
            zrh = {}        # t -> (z, rh) from GRU phase 1

            def bres(m, n, tot):
                return ((m + 1) * n) // tot > (m * n) // tot

            fired = set()

            def pump(step):
                tp, phase = divmod(step, 2)
                if tp < 0 or tp not in aggs or step in fired:
                    return
                fired.add(step)
                if phase == 0:
                    zrh[tp] = emit_gru_zr(tp, aggs[tp])
                else:
                    z, rh = zrh.pop(tp)
                    emit_gru_h(tp, z, rh, aggs.pop(tp))

            NBLK = -(-cfg["shard"] // 128)   # blocks with real atoms
            for t in range(NT):
                agg4 = agp.tile([128, 512, 2], dt.float16, tag="agg4")
                for half in range(BPT // 8):
                    m0 = t * BPT + half * 8
                    if m0 >= NBLK:
                        # all-padding half: aggregation is exactly zero
                        if half * 256 < gtw(t):
                            nc.gpsimd.memset(
                                agg4[:, half * 256:(half + 1) * 256, :], 0.0)
                        pump(2 * (t - POFF) + half)
                        continue
                    xb = sbp3.tile([36, 8, 512], dt.float16, tag="xb",
                                   bufs=TUNE["sbb"])
                    nc.sync.dma_start(
                        out=xb[:],
                        in_=xt4[m0:m0 + 8, :, :].rearrange("m k n -> k m n"))
                    for grp in range(2):
                        gidx = (t * (BPT // 8) + half) * 2 + grp
                        if m0 + grp * 4 >= NBLK:
                            if (half * 2 + grp) * 128 < gtw(t):
                                gsl = slice((half * 2 + grp) * 128,
                                            (half * 2 + grp + 1) * 128)
                                nc.gpsimd.memset(agg4[:, gsl, :], 0.0)
                            continue
                        rl4 = sbp3.tile([128, 4, 512], dt.float16, tag="rl4",
                                        bufs=TUNE["sbb"])
                        # evict pattern: X-groups relu blocks 2,3 on DVE,
                        # Y-groups do both pairs on ACT
                        is_x = bres(gidx, n_x, NGRP)
                        pma = ps1.tile([128, 1024], dt.float32,
                                       space="PSUM", tag="s1a", bufs=2)
                        for bi in range(2):
                            nc.tensor.matmul(
                                pma[:, bi * 512:(bi + 1) * 512], lhsT=wib[:],
                                rhs=xb[:, grp * 4 + bi, :],
                                start=True, stop=True)
                        nc.scalar.activation(
                            rl4[:, 0:2, :], pma[:], ACT.Relu)
                        if is_x:
                            for bi in (2, 3):
                                pmd = ps1.tile([128, 512], dt.float32,
                                               space="PSUM", tag="s1d",
                                               bufs=2)
                                nc.tensor.matmul(pmd[:], lhsT=wib[:],
                                                 rhs=xb[:, grp * 4 + bi, :],
                                                 start=True, stop=True)
                                nc.vector.tensor_scalar(
                                    out=rl4[:, bi, :], in0=pmd[:],
                                    scalar1=0.0, scalar2=None, op0=OP.max)
                        else:
                            pmb = ps1.tile([128, 1024], dt.float32,
                                           space="PSUM", tag="s1a", bufs=2)
                            for bi in range(2):
                                nc.tensor.matmul(
                                    pmb[:, bi * 512:(bi + 1) * 512],
                                    lhsT=wib[:],
                                    rhs=xb[:, grp * 4 + 2 + bi, :],
                                    start=True, stop=True)
                            nc.scalar.activation(
                                rl4[:, 2:4, :], pmb[:], ACT.Relu)
                        v = rl4[:].rearrange("p g (a k) -> p g a k", a=32)
                        q8 = sbp3.tile([128, 4, 32, 8], dt.float16, tag="q8",
                                       bufs=TUNE["sbb"])
                        q8eng = (nc.gpsimd if bres(gidx, n_q8p, NGRP)
                                 else nc.vector)
                        q8eng.tensor_tensor(
                            out=q8[:], in0=v[:, :, :, 0:8],
                            in1=v[:, :, :, 8:16], op=OP.add)
                        q4 = sbp3.tile([128, 4, 32, 4], dt.float16, tag="q4",
                                       bufs=TUNE["sbb"])
                        nc.gpsimd.tensor_tensor(
                            out=q4[:], in0=q8[:, :, :, 0:4],
                            in1=q8[:, :, :, 4:8], op=OP.add)
                        gsl = slice((half * 2 + grp) * 128,
                                    (half * 2 + grp + 1) * 128)
                        nc.gpsimd.tensor_tensor(
                            out=agg4[:, gsl, :].rearrange(
                                "p (g a) s -> p g a s", g=4),
                            in0=q4[:, :, :, 0:2], in1=q4[:, :, :, 2:4],
                            op=OP.add)
                    if t == 0 and half == 0:
                        emit_gw_dma(nc.scalar)
                        nc.scalar.dma_start(out=wia[:], in_=wia4[:])
                    pump(2 * (t - POFF) + half)
                # h0 for tile t
                csl = slice(512 * t, 512 * (t + 1))
                tfb = sbp.tile([32, 512], dt.float16, tag="tfb")
                nc.sync.dma_start(out=tfb[:], in_=tft4[:, csl])
                ph0 = ps1.tile([128, 512], dt.float32, space="PSUM",
                               tag="s1d", bufs=2)
                nc.tensor.matmul(ph0[:], lhsT=wia[:], rhs=tfb[:],
                                 start=True, stop=True)
                nc.scalar.copy(hT[:, csl], ph0[:])
                aggs[t] = agg4
            if NT >= 4:
                for step in range(2 * (NT - POFF), 2 * NT - 4):
                    pump(step)
                pump(2 * (NT - 2))      # zr(NT-2)
                pump(2 * (NT - 1))      # zr(NT-1)
                emit_gru_h_pair([
                    (NT - 2, *zrh.pop(NT - 2), aggs.pop(NT - 2)),
                    (NT - 1, *zrh.pop(NT - 1), aggs.pop(NT - 1))])
            else:
                for step in range(2 * (NT - POFF), 2 * NT):
                    pump(step)
        else:
            # 1024-wide tiles, two-phase software pipeline so PE never
            # convoys on the rh-dependent candidate matmul
            def bgate(t, wi, rhs2):
                w = widths[t]
                pm = psg.tile([128, w], dt.float32, space="PSUM", tag="g",
                              bufs=4)
                for h0 in range(0, w, 512):
                    hw_ = min(512, w - h0)
                    hs = slice(h0, h0 + hw_)
                    csh = slice(offs[t] + h0, offs[t] + h0 + hw_)
                    nc.tensor.matmul(pm[:, hs], lhsT=gw_s(2 * wi),
                                     rhs=agf[:, csh], start=True, stop=False)
                    nc.tensor.matmul(pm[:, hs], lhsT=gw_s(2 * wi + 1),
                                     rhs=rhs2[:, hs], start=False, stop=True)
                return pm

            creal = -(-cfg["shard"] // 4)    # cols with real atoms
            if COLS % 1024 == 0 and COLS // 1024 >= 4:
                n1024 = (COLS - 2048) // 1024
                widths = [512, 512] + [1024] * n1024 + [512, 512]
            else:
                widths = [512] * (COLS // 512)
            # trim trailing all-padding columns (host discards them)
            while sum(widths) - widths[-1] >= creal:
                widths.pop()
            tail = creal - (sum(widths) - widths[-1])
            widths[-1] = min(widths[-1], -(-tail // 8) * 8)
            offs = [sum(widths[:i]) for i in range(len(widths))]
            NTB = len(widths)
            for t in range(NTB):
                cs = slice(offs[t], offs[t] + widths[t])
                nc.sync.dma_start(out=agf[:, cs], in_=aggi[:, cs])
                nc.sync.dma_start(out=hT[:, cs], in_=hi[:, cs])

            def b_zr(t):
                cs = slice(offs[t], offs[t] + widths[t])
                pz = bgate(t, 0, hT[:, cs])
                z = sbp.tile([128, widths[t]], dt.float16, tag="z", bufs=3)
                nc.scalar.activation(z[:], pz[:], ACT.Sigmoid, bias=bw[:, 0:1])
                pr = bgate(t, 1, hT[:, cs])
                r = sbp.tile([128, widths[t]], dt.float16, tag="r")
                nc.scalar.activation(r[:], pr[:], ACT.Sigmoid, bias=bw[:, 1:2])
                rh = sbp.tile([128, widths[t]], dt.float16, tag="rh", bufs=3)
                nc.vector.tensor_tensor(out=rh[:], in0=r[:], in1=hT[:, cs],
                                        op=OP.mult)
                return z, rh

            def b_h(t, z, rh):
                cs = slice(offs[t], offs[t] + widths[t])
                ph = bgate(t, 2, rh[:])
                hc = sbp.tile([128, widths[t]], dt.float16, tag="hc")
                nc.scalar.activation(hc[:], ph[:], ACT.Tanh, bias=bw[:, 2:3])
                d = sbp.tile([128, widths[t]], dt.float16, tag="d")
                nc.vector.tensor_tensor(out=d[:], in0=hc[:], in1=hT[:, cs],
                                        op=OP.subtract)
                zd = sbp.tile([128, widths[t]], dt.float16, tag="zd")
                nc.vector.tensor_tensor(out=zd[:], in0=z[:], in1=d[:],
                                        op=OP.mult)
                nc.vector.tensor_tensor(out=hT[:, cs], in0=hT[:, cs],
                                        in1=zd[:], op=OP.add)
                nc.sync.dma_start(out=out_h[:, cs], in_=hT[:, cs])

            hist = {}
            for t in range(NTB):
                hist[t] = b_zr(t)
                if t >= 2:
                    b_h(t - 2, *hist.pop(t - 2))
            rem = sorted(hist)
            if len(rem) == 2:
                t1, t2 = rem
                z1, rh1 = hist.pop(t1)
                z2, rh2 = hist.pop(t2)
                ph1 = bgate(t1, 2, rh1[:])
                ph2 = bgate(t2, 2, rh2[:])
                cs1 = slice(offs[t1], offs[t1] + widths[t1])
                cs2 = slice(offs[t2], offs[t2] + widths[t2])
                hc1 = sbp.tile([128, widths[t1]], dt.float16, tag="hc")
                nc.scalar.activation(hc1[:], ph1[:], ACT.Tanh, bias=bw[:, 2:3])
                hc2 = sbp.tile([128, widths[t2]], dt.float16, tag="hc")
                nc.scalar.activation(hc2[:], ph2[:], ACT.Tanh, bias=bw[:, 2:3])
                d1 = sbp.tile([128, widths[t1]], dt.float16, tag="d")
                nc.vector.tensor_tensor(out=d1[:], in0=hc1[:], in1=hT[:, cs1],
                                        op=OP.subtract)
                d2 = sbp.tile([128, widths[t2]], dt.float16, tag="d")
                nc.vector.tensor_tensor(out=d2[:], in0=hc2[:], in1=hT[:, cs2],
                                        op=OP.subtract)
                zd1 = sbp.tile([128, widths[t1]], dt.float16, tag="zd")
                nc.vector.tensor_tensor(out=zd1[:], in0=z1[:], in1=d1[:],
                                        op=OP.mult)
                zd2 = sbp.tile([128, widths[t2]], dt.float16, tag="zd")
                nc.vector.tensor_tensor(out=zd2[:], in0=z2[:], in1=d2[:],
                                        op=OP.mult)
                nc.vector.tensor_tensor(out=hT[:, cs1], in0=hT[:, cs1],
                                        in1=zd1[:], op=OP.add)
                nc.sync.dma_start(out=out_h[:, cs1], in_=hT[:, cs1])
                nc.vector.tensor_tensor(out=hT[:, cs2], in0=hT[:, cs2],
                                        in1=zd2[:], op=OP.add)
                nc.sync.dma_start(out=out_h[:, cs2], in_=hT[:, cs2])
            else:
                for t in rem:
                    b_h(t, *hist.pop(t))

    nc.compile()
    _NC_CACHE[key] = nc
    return nc


def _pack4(x, cfg):
    """[SHARD_PAD, 32] row-major -> [128, COLS] 4-packed transposed."""
    return np.ascontiguousarray(
        x.reshape(cfg["cols"], 4, HID).transpose(1, 2, 0)).reshape(
        128, cfg["cols"])


def _unpack4(t4, cfg):
    return np.ascontiguousarray(
        t4.reshape(4, HID, cfg["cols"]).transpose(2, 0, 1)).reshape(-1, HID)


def kernel(**inputs):
    import os

    import ml_dtypes
    from concourse.bass_utils import run_bass_kernel_spmd as _run

    trace = bool(os.environ.get("KTRACE"))
    times = []

    def run_bass_kernel_spmd(nc, maps, core_ids):
        try:
            r = _run(nc, maps, core_ids=core_ids, trace=trace)
        except ModuleNotFoundError:
            # axon NTFF profiling hook unavailable in this image;
            # rerun without tracing rather than failing the launch
            r = _run(nc, maps, core_ids=core_ids, trace=False)
        if r.exec_time_ns:
            times.append(r.exec_time_ns)
        return r

    bf16 = np.float16

    tf = np.asarray(inputs["target_features"], np.float32)
    fdg = np.asarray(inputs["feature_dist_graph"], np.float32)
    rij = np.asarray(inputs["rij_dist_pairs"], np.float32)
    b_scope = np.asarray(inputs["b_scope"], np.int64)
    l_scope = np.asarray(inputs["l_scope"], np.int64)
    su = np.asarray(inputs["scope_update"], np.int64)
    sul = np.asarray(inputs["scope_update_lig"], np.int64)
    W_i_a = np.asarray(inputs["W_i_a"], np.float32)
    W_i_b = np.asarray(inputs["W_i_b"], np.float32)
    W_h = np.asarray(inputs["W_h"], np.float32)
    gW = {k: np.asarray(inputs["gru_W" + k], np.float32) for k in "zrh"}
    gb = {k: np.asarray(inputs["gru_b" + k], np.float32) for k in "zrh"}

    n_atoms = tf.shape[0]
    depth = gW["z"].shape[0]
    cfg = _cfg(n_atoms, depth)
    SHARD, SHARD_PAD, NM1 = cfg["shard"], cfg["shard_pad"], cfg["nm1"]

    valid = b_scope > 0
    pi = np.where(valid, b_scope - 1, 0)
    s1 = np.where(valid, su[pi], n_atoms)   # n_atoms -> zero row
    s2 = np.where(valid, sul[pi], n_atoms)
    ein = np.concatenate([fdg, rij[:, None]], axis=1)
    eidx_g = np.where(valid, pi, -1)

    def b4(w):
        return np.kron(np.eye(4, dtype=np.float32), w)

    def gru_weights(d):
        blocks = []
        for W in (gW["z"][d], gW["r"][d], gW["h"][d]):
            blocks.append(b4(W_h @ W[:HID]))
            blocks.append(b4(W[HID:]))
        gruw = np.concatenate(blocks, axis=0).astype(bf16)
        biasw = np.stack([np.tile(gb[k][d], 4) for k in "zrh"],
                         axis=1).astype(np.float32)
        return gruw, biasw

    wia4 = b4(W_i_a).astype(bf16)
    wib4 = b4(W_i_b).astype(bf16)

    # ---- phase A inputs (stage 1 + h0 + GRU d=0) ----
    gruw0, biasw0 = gru_weights(0)
    in_maps = []
    for c in range(NCORES):
        lo = c * SHARD
        et = np.full((SHARD_PAD, 16), -1, np.int64)
        et[:SHARD] = eidx_g[lo:lo + SHARD]
        m_i = np.arange(NM1)[:, None, None, None]
        u_i = np.arange(4)[None, :, None, None]
        a_i = np.arange(32)[None, None, :, None]
        k_i = np.arange(16)[None, None, None, :]
        pid = et[4 * (32 * m_i + a_i) + u_i, k_i]
        feats = ein[np.clip(pid, 0, None)]
        feats[pid < 0] = 0.0
        xt4 = np.ascontiguousarray(feats.transpose(0, 1, 4, 2, 3)).reshape(
            NM1, 36, 512).astype(bf16)
        tfp = np.zeros((SHARD_PAD, FEAT), np.float32)
        tfp[:SHARD] = tf[lo:lo + SHARD]
        tft4 = np.ascontiguousarray(
            tfp.reshape(cfg["cols"], 4, FEAT).transpose(1, 2, 0)).reshape(
            32, cfg["cols"]).astype(bf16)
        in_maps.append(dict(xt4=xt4, tft4=tft4, gruw=gruw0, biasw=biasw0,
                            wia4=wia4, wib4=wib4))

    ncA = _build("A", cfg)
    res = run_bass_kernel_spmd(ncA, in_maps, core_ids=list(range(NCORES)))

    def collect_h(results):
        h = np.empty((n_atoms, HID), np.float32)
        for c in range(NCORES):
            h[c * SHARD:(c + 1) * SHARD] = _unpack4(
                np.asarray(results[c]["out_h"], np.float32), cfg)[:SHARD]
        return h

    def agg_prime(h):
        # sum of endpoint h rows over valid slots (static composed indices)
        hp = np.concatenate([h, np.zeros((1, HID), np.float32)], axis=0)
        return (hp[s1].sum(axis=1) + hp[s2].sum(axis=1)).astype(np.float32)

    h = collect_h(res.results)
    ncB = _build("B", cfg)
    for d in range(1, depth):
        ap = agg_prime(h)
        gruwd, biaswd = gru_weights(d)
        in_maps = []
        for c in range(NCORES):
            lo = c * SHARD
            apad = np.zeros((SHARD_PAD, HID), np.float32)
            apad[:SHARD] = ap[lo:lo + SHARD]
            hpad = np.zeros((SHARD_PAD, HID), np.float32)
            hpad[:SHARD] = h[lo:lo + SHARD]
            in_maps.append(dict(aggi=_pack4(apad, cfg).astype(bf16),
                                hi=_pack4(hpad, cfg).astype(bf16),
                                gruw=gruwd, biasw=biaswd))
        res = run_bass_kernel_spmd(ncB, in_maps, core_ids=list(range(NCORES)))
        h = collect_h(res.results)

    hp = np.concatenate([np.zeros((1, HID), np.float32), h], axis=0)
    if times:
        print("HW exec time: %d ns (sum of %d launches)"
              % (sum(times), len(times)))
    return hp[l_scope].sum(axis=1).astype(np.float32)


# revision 54
# speedup vs baseline: 1.0120x; 1.0120x over previous
"""PhaGruMPN3 message-passing GNN on 8 TRN2 NeuronCores (Bass/Tile).

Graph/data-parallel sharding (per the sharding hint): atoms are sharded
contiguously across the 8 cores; the per-pair message table is
partitioned per device in consumption order (halo duplication on the
host, which owns all static index gathers), so each core streams its
pair rows sequentially. W_h is folded into the GRU input weights
(associativity), so the 4M-row `em` table is never materialized.

Device-side structure (launch A): the edge relu-matmul runs in a
4-atom-packed transposed layout ([128 partitions = 4 atom blocks x 32
hidden] x [512 columns = 32 atoms x 16 neighbor slots]), all in fp16
(weights in fp16 keep the end-to-end error ~1.7e-3; every DVE op
qualifies for the 2-byte fast path). The neighbor-slot reduction is
split across engines to balance load: PSUM eviction+relu runs on the
Activation engine (1024-wide pairs amortize its fixed access latency)
and on DVE (512-wide singles) - the Pool engine cannot read PSUM - then
one fp16 halving round on DVE (16->8 slots) and two on Pool (8->2)
leave 2 partials per atom whose final sum rides the GRU gate matmuls
as extra PSUM-accumulated terms (the tensor engine has slack). The GRU
is emitted in two software-pipelined phases trailing stage 1 by two
tiles so no engine convoys on the rh-dependent candidate matmul.
Launches B (one per remaining GRU depth) are plain GRU updates on
1024-wide fp16 tiles (512-wide edge tiles shorten pipeline fill/drain,
and the final tile is trimmed to the real atom count) with front-loaded
DMA and a deep PSUM ring. Stage-1 work for all-padding blocks is
skipped outright (their aggregation is exactly zero), the last GRU tile
of both launches is trimmed to the columns holding real atoms, preamble
weight DMAs issue from the Activation/Pool queues so the first edge
chunk is not serialized behind them on SP, and the last two tiles' GRU
tails are interleaved to hide the serial DVE chain during drain. Between launches
the host applies the composed static-index gather-sum
(b_scope o scope_update) and re-shards.
"""

import sys

sys.path.insert(0, "/opt/trn_rl_repo")

import numpy as np

HID = 32
FEAT = 8
NCORES = 8


def _cfg(n_atoms, depth):
    assert n_atoms % NCORES == 0
    shard = n_atoms // NCORES
    shard_pad = -(-shard // 2048) * 2048
    cols = shard_pad // 4
    return dict(n_atoms=n_atoms, depth=depth, shard=shard, shard_pad=shard_pad,
                cols=cols, nt_gru=cols // 512, nm1=shard_pad // 128)


_NC_CACHE = {}


def _build(kind, cfg):
    """kind 'A': stage1 + h0 + GRU(d=0) -> h1. kind 'B': GRU(one depth)."""
    key = (kind, tuple(sorted(cfg.items())))
    if key in _NC_CACHE:
        return _NC_CACHE[key]
    import concourse.bacc as bacc
    import concourse.tile as tile
    from concourse import mybir

    dt = mybir.dt
    AX = mybir.AxisListType
    OP = mybir.AluOpType
    ACT = mybir.ActivationFunctionType

    COLS = cfg["cols"]
    NT = cfg["nt_gru"]
    NM1 = cfg["nm1"]
    BPT = NM1 // NT        # stage-1 blocks per GRU tile (16)

    nc = bacc.Bacc("TRN2", target_bir_lowering=False, debug=False,
                   enable_asserts=False, num_devices=NCORES)

    if kind == "A":
        xt4 = nc.dram_tensor("xt4", [NM1, 36, 512], dt.float16,
                             kind="ExternalInput")
        tft4 = nc.dram_tensor("tft4", [32, COLS], dt.float16,
                              kind="ExternalInput")
        wia4 = nc.dram_tensor("wia4", [32, 128], dt.float16,
                              kind="ExternalInput")
        wib4 = nc.dram_tensor("wib4", [36, 128], dt.float16,
                              kind="ExternalInput")
    else:
        aggi = nc.dram_tensor("aggi", [128, COLS], dt.float16,
                              kind="ExternalInput")
        hi = nc.dram_tensor("hi", [128, COLS], dt.float16,
                            kind="ExternalInput")
    gruw = nc.dram_tensor("gruw", [128, 6 * 128], dt.float16,
                          kind="ExternalInput")
    biasw = nc.dram_tensor("biasw", [128, 3], dt.float32, kind="ExternalInput")
    out_h = nc.dram_tensor("out_h", [128, COLS], dt.float16,
                           kind="ExternalOutput")

    with tile.TileContext(nc) as tc, \
         tc.tile_pool(name="persist", bufs=1) as pp, \
         tc.tile_pool(name="ps1", bufs=3, space="PSUM") as ps1, \
         tc.tile_pool(name="psg", bufs=2, space="PSUM") as psg, \
         tc.tile_pool(name="sb", bufs=2) as sbp, \
         tc.tile_pool(name="sb3", bufs=4) as sbp3, \
         tc.tile_pool(name="agp", bufs=4) as agp, \
         nc.allow_low_precision(reason="bf16 gnn pipeline"):

        gw = pp.tile([128, 6 * 128], dt.float16, name="gw")

        def emit_gw_dma(eng):
            eng.dma_start(out=gw[:], in_=gruw[:])

        bw = pp.tile([128, 3], dt.float32, name="bw")
        if kind == "B":
            emit_gw_dma(nc.sync)
            nc.scalar.dma_start(out=bw[:], in_=biasw[:])
        else:
            nc.gpsimd.dma_start(out=bw[:], in_=biasw[:])
        hT = pp.tile([128, COLS], dt.float16, name="hT")

        def gw_s(i):
            return gw[:, i * 128:(i + 1) * 128]

        if kind == "A":
            wib = pp.tile([36, 128], dt.float16, name="wib")
            nc.sync.dma_start(out=wib[:], in_=wib4[:])
            wia = pp.tile([32, 128], dt.float16, name="wia")
        else:
            agf = pp.tile([128, COLS], dt.float16, name="agf")

        CREAL = -(-cfg["shard"] // 4)    # cols holding real atoms

        def gtw(t):
            # GRU width of tile t, trimmed to real columns on the last tile
            if kind != "A":
                return 512
            return max(8, min(512, -(-(CREAL - 512 * t) // 8) * 8))

        def gate(cs, wi, rhs2, agg4, w=512):
            pm = psg.tile([128, w], dt.float32, space="PSUM", tag="g",
                          bufs=2)
            if kind == "A":
                for s in range(2):
                    nc.tensor.matmul(pm[:], lhsT=gw_s(2 * wi),
                                     rhs=agg4[:, :w, s],
                                     start=(s == 0), stop=False)
            else:
                nc.tensor.matmul(pm[:], lhsT=gw_s(2 * wi),
                                 rhs=agf[:, cs], start=True, stop=False)
            nc.tensor.matmul(pm[:], lhsT=gw_s(2 * wi + 1), rhs=rhs2,
                             start=False, stop=True)
            return pm

        def emit_gru_zr(t, agg4=None):
            """GRU phase 1 on tile t: gates z, r and rh = r*h."""
            w = gtw(t)
            cs = slice(512 * t, 512 * t + w)
            pz = gate(cs, 0, hT[:, cs], agg4, w)
            z = sbp.tile([128, w], dt.float16, tag="z")
            nc.scalar.activation(z[:], pz[:], ACT.Sigmoid, bias=bw[:, 0:1])
            pr = gate(cs, 1, hT[:, cs], agg4, w)
            r = sbp.tile([128, w], dt.float16, tag="r")
            nc.scalar.activation(r[:], pr[:], ACT.Sigmoid, bias=bw[:, 1:2])
            rh = sbp.tile([128, w], dt.float16, tag="rh")
            nc.vector.tensor_tensor(out=rh[:], in0=r[:], in1=hT[:, cs],
                                    op=OP.mult)
            return z, rh

        def emit_gru_h_pair(ts_):
            """Interleave the final tiles' phase-2 chains to hide the
            serial DVE tail during drain."""
            phs, hcs, ds, zds = [], [], [], []
            for t, z, rh, agg4 in ts_:
                w = gtw(t)
                cs = slice(512 * t, 512 * t + w)
                phs.append((cs, w, gate(cs, 2, rh[:], agg4, w)))
            for cs, w, ph in phs:
                hc = sbp.tile([128, w], dt.float16, tag="hc")
                nc.scalar.activation(hc[:], ph[:], ACT.Tanh, bias=bw[:, 2:3])
                hcs.append((cs, w, hc))
            for cs, w, hc in hcs:
                d = sbp.tile([128, w], dt.float16, tag="d")
                nc.vector.tensor_tensor(out=d[:], in0=hc[:], in1=hT[:, cs],
                                        op=OP.subtract)
                ds.append((cs, w, d))
            for (t, z, rh, agg4), (cs, w, d) in zip(ts_, ds):
                zd = sbp.tile([128, w], dt.float16, tag="zd")
                nc.vector.tensor_tensor(out=zd[:], in0=z[:], in1=d[:],
                                        op=OP.mult)
                zds.append((cs, zd))
            for cs, zd in zds:
                nc.vector.tensor_tensor(out=hT[:, cs], in0=hT[:, cs],
                                        in1=zd[:], op=OP.add)
                nc.sync.dma_start(out=out_h[:, cs], in_=hT[:, cs])

        def emit_gru_h(t, z, rh, agg4=None, tteng=None):
            """GRU phase 2 on tile t: candidate hc and the h update."""
            w = gtw(t)
            te = tteng or nc.vector
            cs = slice(512 * t, 512 * t + w)
            ph = gate(cs, 2, rh[:], agg4, w)
            hc = sbp.tile([128, w], dt.float16, tag="hc")
            nc.scalar.activation(hc[:], ph[:], ACT.Tanh, bias=bw[:, 2:3])
            d = sbp.tile([128, w], dt.float16, tag="d")
            nc.vector.tensor_tensor(out=d[:], in0=hc[:], in1=hT[:, cs],
                                    op=OP.subtract)
            zd = sbp.tile([128, w], dt.float16, tag="zd")
            te.tensor_tensor(out=zd[:], in0=z[:], in1=d[:],
                             op=OP.mult)
            te.tensor_tensor(out=hT[:, cs], in0=hT[:, cs], in1=zd[:],
                             op=OP.add)
            nc.sync.dma_start(out=out_h[:, cs], in_=hT[:, cs])

        if kind == "A":
            import os
            TUNE = dict(n_x=48, n_q8p=0, sbb=4, poff=2)
            for kv in os.environ.get("KTUNE", "").split(","):
                if ":" in kv:
                    k, v = kv.split(":")
                    TUNE[k] = int(v)
            NGRP = NM1 // 4
            n_x = (NGRP * TUNE["n_x"]) // 64
            n_q8p = (NGRP * TUNE["n_q8p"]) // 64
            POFF = TUNE["poff"]
            aggs = {}       # t -> agg4 tile (stage 1 emitted)
            zrh = {}        # t -> (z, rh) from GRU phase 1

            def bres(m, n, tot):
                return ((m + 1) * n) // tot > (m * n) // tot

            fired = set()

            def pump(step):
                tp, phase = divmod(step, 2)
                if tp < 0 or tp not in aggs or step in fired:
                    return
                fired.add(step)
                if phase == 0:
                    zrh[tp] = emit_gru_zr(tp, aggs[tp])
                else:
                    z, rh = zrh.pop(tp)
                    emit_gru_h(tp, z, rh, aggs.pop(tp))

            NBLK = -(-cfg["shard"] // 128)   # blocks with real atoms
            for t in range(NT):
                agg4 = agp.tile([128, 512, 2], dt.float16, tag="agg4")
                for half in range(BPT // 8):
                    m0 = t * BPT + half * 8
                    if m0 >= NBLK:
                        # all-padding half: aggregation is exactly zero
                        if half * 256 < gtw(t):
                            nc.gpsimd.memset(
                                agg4[:, half * 256:(half + 1) * 256, :], 0.0)
                        pump(2 * (t - POFF) + half)
                        continue
                    xb = sbp3.tile([36, 8, 512], dt.float16, tag="xb",
                                   bufs=TUNE["sbb"])
                    nc.sync.dma_start(
                        out=xb[:],
                        in_=xt4[m0:m0 + 8, :, :].rearrange("m k n -> k m n"))
                    for grp in range(2):
                        gidx = (t * (BPT // 8) + half) * 2 + grp
                        if m0 + grp * 4 >= NBLK:
                            if (half * 2 + grp) * 128 < gtw(t):
                                gsl = slice((half * 2 + grp) * 128,
                                            (half * 2 + grp + 1) * 128)
                                nc.gpsimd.memset(agg4[:, gsl, :], 0.0)
                            continue
                        rl4 = sbp3.tile([128, 4, 512], dt.float16, tag="rl4",
                                        bufs=TUNE["sbb"])
                        # evict pattern: X-groups relu blocks 2,3 on DVE,
                        # Y-groups do both pairs on ACT
                        is_x = bres(gidx, n_x, NGRP)
                        pma = ps1.tile([128, 1024], dt.float32,
                                       space="PSUM", tag="s1a", bufs=2)
                        for bi in range(2):
                            nc.tensor.matmul(
                                pma[:, bi * 512:(bi + 1) * 512], lhsT=wib[:],
                                rhs=xb[:, grp * 4 + bi, :],
                                start=True, stop=True)
                        nc.scalar.activation(
                            rl4[:, 0:2, :], pma[:], ACT.Relu)
                        if is_x:
                            for bi in (2, 3):
                                pmd = ps1.tile([128, 512], dt.float32,
                                               space="PSUM", tag="s1d",
                                               bufs=2)
                                nc.tensor.matmul(pmd[:], lhsT=wib[:],
                                                 rhs=xb[:, grp * 4 + bi, :],
                                                 start=True, stop=True)
                                nc.vector.tensor_scalar(
                                    out=rl4[:, bi, :], in0=pmd[:],
                                    scalar1=0.0, scalar2=None, op0=OP.max)
                        else:
                            pmb = ps1.tile([128, 1024], dt.float32,
                                           space="PSUM", tag="s1a", bufs=2)
                            for bi in range(2):
                                nc.tensor.matmul(
                                    pmb[:, bi * 512:(bi + 1) * 512],
                                    lhsT=wib[:],
                                    rhs=xb[:, grp * 4 + 2 + bi, :],
                                    start=True, stop=True)
                            nc.scalar.activation(
                                rl4[:, 2:4, :], pmb[:], ACT.Relu)
                        v = rl4[:].rearrange("p g (a k) -> p g a k", a=32)
                        q8 = sbp3.tile([128, 4, 32, 8], dt.float16, tag="q8",
                                       bufs=TUNE["sbb"])
                        q8eng = (nc.gpsimd if bres(gidx, n_q8p, NGRP)
                                 else nc.vector)
                        q8eng.tensor_tensor(
                            out=q8[:], in0=v[:, :, :, 0:8],
                            in1=v[:, :, :, 8:16], op=OP.add)
                        q4 = sbp3.tile([128, 4, 32, 4], dt.float16, tag="q4",
                                       bufs=TUNE["sbb"])
                        nc.gpsimd.tensor_tensor(
                            out=q4[:], in0=q8[:, :, :, 0:4],
                            in1=q8[:, :, :, 4:8], op=OP.add)
                        gsl = slice((half * 2 + grp) * 128,
                                    (half * 2 + grp + 1) * 128)
                        nc.gpsimd.tensor_tensor(
                            out=agg4[:, gsl, :].rearrange(
                                "p (g a) s -> p g a s", g=4),
                            in0=q4[:, :, :, 0:2], in1=q4[:, :, :, 2:4],
                            op=OP.add)
                    if t == 0 and half == 0:
                        emit_gw_dma(nc.scalar)
                        nc.scalar.dma_start(out=wia[:], in_=wia4[:])
                    pump(2 * (t - POFF) + half)
                # h0 for tile t
                csl = slice(512 * t, 512 * (t + 1))
                tfb = sbp.tile([32, 512], dt.float16, tag="tfb")
                nc.sync.dma_start(out=tfb[:], in_=tft4[:, csl])
                ph0 = ps1.tile([128, 512], dt.float32, space="PSUM",
                               tag="s1d", bufs=2)
                nc.tensor.matmul(ph0[:], lhsT=wia[:], rhs=tfb[:],
                                 start=True, stop=True)
                nc.scalar.copy(hT[:, csl], ph0[:])
                aggs[t] = agg4
            if NT >= 4:
                for step in range(2 * (NT - POFF), 2 * NT - 4):
                    pump(step)
                pump(2 * (NT - 2))      # zr(NT-2)
                pump(2 * (NT - 1))      # zr(NT-1)
                emit_gru_h_pair([
                    (NT - 2, *zrh.pop(NT - 2), aggs.pop(NT - 2)),
                    (NT - 1, *zrh.pop(NT - 1), aggs.pop(NT - 1))])
            else:
                for step in range(2 * (NT - POFF), 2 * NT):
                    pump(step)
        else:
            # 1024-wide tiles, two-phase software pipeline so PE never
            # convoys on the rh-dependent candidate matmul
            def bgate(t, wi, rhs2):
                w = widths[t]
                pm = psg.tile([128, w], dt.float32, space="PSUM", tag="g",
                              bufs=4)
                for h0 in range(0, w, 512):
                    hw_ = min(512, w - h0)
                    hs = slice(h0, h0 + hw_)
                    csh = slice(offs[t] + h0, offs[t] + h0 + hw_)
                    nc.tensor.matmul(pm[:, hs], lhsT=gw_s(2 * wi),
                                     rhs=agf[:, csh], start=True, stop=False)
                    nc.tensor.matmul(pm[:, hs], lhsT=gw_s(2 * wi + 1),
                                     rhs=rhs2[:, hs], start=False, stop=True)
                return pm

            creal = -(-cfg["shard"] // 4)    # cols with real atoms
            if COLS % 1024 == 0 and COLS // 1024 >= 4:
                n1024 = (COLS - 2048) // 1024
                widths = [512, 512] + [1024] * n1024 + [512, 512]
            else:
                widths = [512] * (COLS // 512)
            # trim trailing all-padding columns (host discards them)
            while sum(widths) - widths[-1] >= creal:
                widths.pop()
            tail = creal - (sum(widths) - widths[-1])
            widths[-1] = min(widths[-1], -(-tail // 8) * 8)
            offs = [sum(widths[:i]) for i in range(len(widths))]
            NTB = len(widths)
            for t in range(NTB):
                cs = slice(offs[t], offs[t] + widths[t])
                nc.sync.dma_start(out=agf[:, cs], in_=aggi[:, cs])
                nc.sync.dma_start(out=hT[:, cs], in_=hi[:, cs])

            def b_zr(t):
                cs = slice(offs[t], offs[t] + widths[t])
                pz = bgate(t, 0, hT[:, cs])
                z = sbp.tile([128, widths[t]], dt.float16, tag="z", bufs=3)
                nc.scalar.activation(z[:], pz[:], ACT.Sigmoid, bias=bw[:, 0:1])
                pr = bgate(t, 1, hT[:, cs])
                r = sbp.tile([128, widths[t]], dt.float16, tag="r")
                nc.scalar.activation(r[:], pr[:], ACT.Sigmoid, bias=bw[:, 1:2])
                rh = sbp.tile([128, widths[t]], dt.float16, tag="rh", bufs=3)
                nc.vector.tensor_tensor(out=rh[:], in0=r[:], in1=hT[:, cs],
                                        op=OP.mult)
                return z, rh

            def b_h(t, z, rh):
                cs = slice(offs[t], offs[t] + widths[t])
                ph = bgate(t, 2, rh[:])
                hc = sbp.tile([128, widths[t]], dt.float16, tag="hc")
                nc.scalar.activation(hc[:], ph[:], ACT.Tanh, bias=bw[:, 2:3])
                d = sbp.tile([128, widths[t]], dt.float16, tag="d")
                nc.vector.tensor_tensor(out=d[:], in0=hc[:], in1=hT[:, cs],
                                        op=OP.subtract)
                zd = sbp.tile([128, widths[t]], dt.float16, tag="zd")
                nc.vector.tensor_tensor(out=zd[:], in0=z[:], in1=d[:],
                                        op=OP.mult)
                nc.vector.tensor_tensor(out=hT[:, cs], in0=hT[:, cs],
                                        in1=zd[:], op=OP.add)
                nc.sync.dma_start(out=out_h[:, cs], in_=hT[:, cs])

            hist = {}
            for t in range(NTB):
                hist[t] = b_zr(t)
                if t >= 2:
                    b_h(t - 2, *hist.pop(t - 2))
            rem = sorted(hist)
            if len(rem) == 2:
                t1, t2 = rem
                z1, rh1 = hist.pop(t1)
                z2, rh2 = hist.pop(t2)
                ph1 = bgate(t1, 2, rh1[:])
                ph2 = bgate(t2, 2, rh2[:])
                cs1 = slice(offs[t1], offs[t1] + widths[t1])
                cs2 = slice(offs[t2], offs[t2] + widths[t2])
                hc1 = sbp.tile([128, widths[t1]], dt.float16, tag="hc")
                nc.scalar.activation(hc1[:], ph1[:], ACT.Tanh, bias=bw[:, 2:3])
                hc2 = sbp.tile([128, widths[t2]], dt.float16, tag="hc")
                nc.scalar.activation(hc2[:], ph2[:], ACT.Tanh, bias=bw[:, 2:3])
                d1 = sbp.tile([128, widths[t1]], dt.float16, tag="d")
                nc.vector.tensor_tensor(out=d1[:], in0=hc1[:], in1=hT[:, cs1],
                                        op=OP.subtract)
                d2 = sbp.tile([128, widths[t2]], dt.float16, tag="d")
                nc.vector.tensor_tensor(out=d2[:], in0=hc2[:], in1=hT[:, cs2],
                                        op=OP.subtract)
                zd1 = sbp.tile([128, widths[t1]], dt.float16, tag="zd")
                nc.vector.tensor_tensor(out=zd1[:], in0=z1[:], in1=d1[:],
                                        op=OP.mult)
                zd2 = sbp.tile([128, widths[t2]], dt.float16, tag="zd")
                nc.vector.tensor_tensor(out=zd2[:], in0=z2[:], in1=d2[:],
                                        op=OP.mult)
                nc.vector.tensor_tensor(out=hT[:, cs1], in0=hT[:, cs1],
                                        in1=zd1[:], op=OP.add)
                nc.sync.dma_start(out=out_h[:, cs1], in_=hT[:, cs1])
                nc.vector.tensor_tensor(out=hT[:, cs2], in0=hT[:, cs2],
                                        in1=zd2[:], op=OP.add)
                nc.sync.dma_start(out=out_h[:, cs2], in_=hT[:, cs2])
            else:
                for t in rem:
                    b_h(t, *hist.pop(t))

    nc.compile()
    _NC_CACHE[key] = nc
    return nc


def _pack4(x, cfg):
    """[SHARD_PAD, 32] row-major -> [128, COLS] 4-packed transposed."""
    return np.ascontiguousarray(
        x.reshape(cfg["cols"], 4, HID).transpose(1, 2, 0)).reshape(
        128, cfg["cols"])


def _unpack4(t4, cfg):
    return np.ascontiguousarray(
        t4.reshape(4, HID, cfg["cols"]).transpose(2, 0, 1)).reshape(-1, HID)


def kernel(**inputs):
    import os

    import ml_dtypes
    from concourse.bass_utils import run_bass_kernel_spmd as _run

    trace = bool(os.environ.get("KTRACE"))
    times = []

    def run_bass_kernel_spmd(nc, maps, core_ids):
        try:
            r = _run(nc, maps, core_ids=core_ids, trace=trace)
        except ModuleNotFoundError:
            # axon NTFF profiling hook unavailable in this image;
            # rerun without tracing rather than failing the launch
            r = _run(nc, maps, core_ids=core_ids, trace=False)
        if r.exec_time_ns:
            times.append(r.exec_time_ns)
        return r

    bf16 = np.float16

    tf = np.asarray(inputs["target_features"], np.float32)
    fdg = np.asarray(inputs["feature_dist_graph"], np.float32)
    rij = np.asarray(inputs["rij_dist_pairs"], np.float32)
    b_scope = np.asarray(inputs["b_scope"], np.int64)
    l_scope = np.asarray(inputs["l_scope"], np.int64)
    su = np.asarray(inputs["scope_update"], np.int64)
    sul = np.asarray(inputs["scope_update_lig"], np.int64)
    W_i_a = np.asarray(inputs["W_i_a"], np.float32)
    W_i_b = np.asarray(inputs["W_i_b"], np.float32)
    W_h = np.asarray(inputs["W_h"], np.float32)
    gW = {k: np.asarray(inputs["gru_W" + k], np.float32) for k in "zrh"}
    gb = {k: np.asarray(inputs["gru_b" + k], np.float32) for k in "zrh"}

    n_atoms = tf.shape[0]
    depth = gW["z"].shape[0]
    cfg = _cfg(n_atoms, depth)
    SHARD, SHARD_PAD, NM1 = cfg["shard"], cfg["shard_pad"], cfg["nm1"]

    valid = b_scope > 0
    pi = np.where(valid, b_scope - 1, 0)
    s1 = np.where(valid, su[pi], n_atoms)   # n_atoms -> zero row
    s2 = np.where(valid, sul[pi], n_atoms)
    ein = np.concatenate([fdg, rij[:, None]], axis=1)
    eidx_g = np.where(valid, pi, -1)

    def b4(w):
        return np.kron(np.eye(4, dtype=np.float32), w)

    def gru_weights(d):
        blocks = []
        for W in (gW["z"][d], gW["r"][d], gW["h"][d]):
            blocks.append(b4(W_h @ W[:HID]))
            blocks.append(b4(W[HID:]))
        gruw = np.ascontiguousarray(
            np.stack(blocks, axis=1).transpose(0, 1, 2)  # [128,6,128]
            .reshape(128, 6 * 128)).astype(bf16)
        biasw = np.stack([np.tile(gb[k][d], 4) for k in "zrh"],
                         axis=1).astype(np.float32)
        return gruw, biasw

    wia4 = b4(W_i_a).astype(bf16)
    wib4 = b4(W_i_b).astype(bf16)

    # ---- phase A inputs (stage 1 + h0 + GRU d=0) ----
    gruw0, biasw0 = gru_weights(0)
    in_maps = []
    for c in range(NCORES):
        lo = c * SHARD
        et = np.full((SHARD_PAD, 16), -1, np.int64)
        et[:SHARD] = eidx_g[lo:lo + SHARD]
        m_i = np.arange(NM1)[:, None, None, None]
        u_i = np.arange(4)[None, :, None, None]
        a_i = np.arange(32)[None, None, :, None]
        k_i = np.arange(16)[None, None, None, :]
        pid = et[4 * (32 * m_i + a_i) + u_i, k_i]
        feats = ein[np.clip(pid, 0, None)]
        feats[pid < 0] = 0.0
        xt4 = np.ascontiguousarray(feats.transpose(0, 1, 4, 2, 3)).reshape(
            NM1, 36, 512).astype(bf16)
        tfp = np.zeros((SHARD_PAD, FEAT), np.float32)
        tfp[:SHARD] = tf[lo:lo + SHARD]
        tft4 = np.ascontiguousarray(
            tfp.reshape(cfg["cols"], 4, FEAT).transpose(1, 2, 0)).reshape(
            32, cfg["cols"]).astype(bf16)
        in_maps.append(dict(xt4=xt4, tft4=tft4, gruw=gruw0, biasw=biasw0,
                            wia4=wia4, wib4=wib4))

    ncA = _build("A", cfg)
    res = run_bass_kernel_spmd(ncA, in_maps, core_ids=list(range(NCORES)))

    def collect_h(results):
        h = np.empty((n_atoms, HID), np.float32)
        for c in range(NCORES):
            h[c * SHARD:(c + 1) * SHARD] = _unpack4(
                np.asarray(results[c]["out_h"], np.float32), cfg)[:SHARD]
        return h

    def agg_prime(h):
        # sum of endpoint h rows over valid slots (static composed indices)
        hp = np.concatenate([h, np.zeros((1, HID), np.float32)], axis=0)
        return (hp[s1].sum(axis=1) + hp[s2].sum(axis=1)).astype(np.float32)

    h = collect_h(res.results)
    ncB = _build("B", cfg)
    for d in range(1, depth):
        ap = agg_prime(h)
        gruwd, biaswd = gru_weights(d)
        in_maps = []
        for c in range(NCORES):
            lo = c * SHARD
            apad = np.zeros((SHARD_PAD, HID), np.float32)
            apad[:SHARD] = ap[lo:lo + SHARD]
            hpad = np.zeros((SHARD_PAD, HID), np.float32)
            hpad[:SHARD] = h[lo:lo + SHARD]
            in_maps.append(dict(aggi=_pack4(apad, cfg).astype(bf16),
                                hi=_pack4(hpad, cfg).astype(bf16),
                                gruw=gruwd, biasw=biaswd))
        res = run_bass_kernel_spmd(ncB, in_maps, core_ids=list(range(NCORES)))
        h = collect_h(res.results)

    hp = np.concatenate([np.zeros((1, HID), np.float32), h], axis=0)
    if times:
        print("HW exec time: %d ns (sum of %d launches)"
              % (sum(times), len(times)))
    return hp[l_scope].sum(axis=1).astype(np.float32)


# revision 61
# speedup vs baseline: 1.0139x; 1.0019x over previous
"""PhaGruMPN3 message-passing GNN on 8 TRN2 NeuronCores (Bass/Tile).

Graph/data-parallel sharding (per the sharding hint): atoms are sharded
contiguously across the 8 cores; the per-pair message table is
partitioned per device in consumption order (halo duplication on the
host, which owns all static index gathers), so each core streams its
pair rows sequentially. W_h is folded into the GRU input weights
(associativity), so the 4M-row `em` table is never materialized.

Device-side structure (launch A): the edge relu-matmul runs in a
4-atom-packed transposed layout ([128 partitions = 4 atom blocks x 32
hidden] x [512 columns = 32 atoms x 16 neighbor slots]), all in fp16
(weights in fp16 keep the end-to-end error ~1.7e-3; every DVE op
qualifies for the 2-byte fast path). The neighbor-slot reduction is
split across engines to balance load: PSUM eviction+relu runs on the
Activation engine (1024-wide pairs amortize its fixed access latency)
and on DVE (512-wide singles) - the Pool engine cannot read PSUM - then
one fp16 halving round on DVE (16->8 slots) and two on Pool (8->2)
leave 2 partials per atom whose final sum rides the GRU gate matmuls
as extra PSUM-accumulated terms (the tensor engine has slack). The GRU
is emitted in two software-pipelined phases trailing stage 1 by two
tiles so no engine convoys on the rh-dependent candidate matmul.
Launches B (one per remaining GRU depth) are plain GRU updates on
1024-wide fp16 tiles (512-wide edge tiles shorten pipeline fill/drain,
and the final tile is trimmed to the real atom count) with front-loaded
DMA and a deep PSUM ring. Stage-1 work for all-padding blocks is
skipped outright (their aggregation is exactly zero), the last GRU tile
of both launches is trimmed to the columns holding real atoms, preamble
weight DMAs issue from the Activation/Pool queues so the first edge
chunk is not serialized behind them on SP, and the last two tiles' GRU
tails are interleaved to hide the serial DVE chain during drain. Between launches
the host applies the composed static-index gather-sum
(b_scope o scope_update) and re-shards.
"""

import sys

sys.path.insert(0, "/opt/trn_rl_repo")

import numpy as np

HID = 32
FEAT = 8
NCORES = 8


def _cfg(n_atoms, depth):
    assert n_atoms % NCORES == 0
    shard = n_atoms // NCORES
    shard_pad = -(-shard // 2048) * 2048
    cols = shard_pad // 4
    return dict(n_atoms=n_atoms, depth=depth, shard=shard, shard_pad=shard_pad,
                cols=cols, nt_gru=cols // 512, nm1=shard_pad // 128)


_NC_CACHE = {}


def _build(kind, cfg):
    """kind 'A': stage1 + h0 + GRU(d=0) -> h1. kind 'B': GRU(one depth)."""
    key = (kind, tuple(sorted(cfg.items())))
    if key in _NC_CACHE:
        return _NC_CACHE[key]
    import concourse.bacc as bacc
    import concourse.tile as tile
    from concourse import mybir

    dt = mybir.dt
    AX = mybir.AxisListType
    OP = mybir.AluOpType
    ACT = mybir.ActivationFunctionType

    COLS = cfg["cols"]
    NT = cfg["nt_gru"]
    NM1 = cfg["nm1"]
    BPT = NM1 // NT        # stage-1 blocks per GRU tile (16)

    nc = bacc.Bacc("TRN2", target_bir_lowering=False, debug=False,
                   enable_asserts=False, num_devices=NCORES)

    if kind == "A":
        xt4 = nc.dram_tensor("xt4", [NM1, 36, 512], dt.float16,
                             kind="ExternalInput")
        tft4 = nc.dram_tensor("tft4", [32, COLS], dt.float16,
                              kind="ExternalInput")
        wia4 = nc.dram_tensor("wia4", [32, 128], dt.float16,
                              kind="ExternalInput")
        wib4 = nc.dram_tensor("wib4", [36, 128], dt.float16,
                              kind="ExternalInput")
    else:
        aggi = nc.dram_tensor("aggi", [128, COLS], dt.float16,
                              kind="ExternalInput")
        hi = nc.dram_tensor("hi", [128, COLS], dt.float16,
                            kind="ExternalInput")
    gruw = nc.dram_tensor("gruw", [128, 6 * 128], dt.float16,
                          kind="ExternalInput")
    biasw = nc.dram_tensor("biasw", [128, 3], dt.float32, kind="ExternalInput")
    out_h = nc.dram_tensor("out_h", [128, COLS], dt.float16,
                           kind="ExternalOutput")

    with tile.TileContext(nc) as tc, \
         tc.tile_pool(name="persist", bufs=1) as pp, \
         tc.tile_pool(name="ps1", bufs=3, space="PSUM") as ps1, \
         tc.tile_pool(name="psg", bufs=2, space="PSUM") as psg, \
         tc.tile_pool(name="sb", bufs=3) as sbp, \
         tc.tile_pool(name="sb3", bufs=4) as sbp3, \
         tc.tile_pool(name="agp", bufs=5) as agp, \
         nc.allow_low_precision(reason="bf16 gnn pipeline"):

        gw = pp.tile([128, 6 * 128], dt.float16, name="gw")

        def emit_gw_dma(eng):
            eng.dma_start(out=gw[:], in_=gruw[:])

        bw = pp.tile([128, 3], dt.float32, name="bw")
        if kind == "B":
            emit_gw_dma(nc.sync)
            nc.scalar.dma_start(out=bw[:], in_=biasw[:])
        else:
            nc.gpsimd.dma_start(out=bw[:], in_=biasw[:])
        hT = pp.tile([128, COLS], dt.float16, name="hT")

        def gw_s(i):
            return gw[:, i * 128:(i + 1) * 128]

        if kind == "A":
            wib = pp.tile([36, 128], dt.float16, name="wib")
            nc.sync.dma_start(out=wib[:], in_=wib4[:])
            wia = pp.tile([32, 128], dt.float16, name="wia")
        else:
            agf = pp.tile([128, COLS], dt.float16, name="agf")

        CREAL = -(-cfg["shard"] // 4)    # cols holding real atoms

        def gtw(t):
            # GRU width of tile t, trimmed to real columns on the last tile
            if kind != "A":
                return 512
            return max(8, min(512, -(-(CREAL - 512 * t) // 8) * 8))

        def gate(cs, wi, rhs2, agg4, w=512):
            pm = psg.tile([128, w], dt.float32, space="PSUM", tag="g",
                          bufs=2)
            if kind == "A":
                for s in range(2):
                    nc.tensor.matmul(pm[:], lhsT=gw_s(2 * wi),
                                     rhs=agg4[:, :w, s],
                                     start=(s == 0), stop=False)
            else:
                nc.tensor.matmul(pm[:], lhsT=gw_s(2 * wi),
                                 rhs=agf[:, cs], start=True, stop=False)
            nc.tensor.matmul(pm[:], lhsT=gw_s(2 * wi + 1), rhs=rhs2,
                             start=False, stop=True)
            return pm

        def emit_gru_zr(t, agg4=None):
            """GRU phase 1 on tile t: gates z, r and rh = r*h."""
            w = gtw(t)
            cs = slice(512 * t, 512 * t + w)
            pz = gate(cs, 0, hT[:, cs], agg4, w)
            z = sbp.tile([128, w], dt.float16, tag="z")
            nc.scalar.activation(z[:], pz[:], ACT.Sigmoid, bias=bw[:, 0:1])
            pr = gate(cs, 1, hT[:, cs], agg4, w)
            r = sbp.tile([128, w], dt.float16, tag="r")
            nc.scalar.activation(r[:], pr[:], ACT.Sigmoid, bias=bw[:, 1:2])
            rh = sbp.tile([128, w], dt.float16, tag="rh")
            nc.vector.tensor_tensor(out=rh[:], in0=r[:], in1=hT[:, cs],
                                    op=OP.mult)
            return z, rh

        def emit_gru_h_pair(ts_):
            """Interleave the final tiles' phase-2 chains to hide the
            serial DVE tail during drain."""
            phs, hcs, ds, zds = [], [], [], []
            for t, z, rh, agg4 in ts_:
                w = gtw(t)
                cs = slice(512 * t, 512 * t + w)
                phs.append((cs, w, gate(cs, 2, rh[:], agg4, w)))
            for cs, w, ph in phs:
                hc = sbp.tile([128, w], dt.float16, tag="hc")
                nc.scalar.activation(hc[:], ph[:], ACT.Tanh, bias=bw[:, 2:3])
                hcs.append((cs, w, hc))
            for cs, w, hc in hcs:
                d = sbp.tile([128, w], dt.float16, tag="d")
                nc.vector.tensor_tensor(out=d[:], in0=hc[:], in1=hT[:, cs],
                                        op=OP.subtract)
                ds.append((cs, w, d))
            for (t, z, rh, agg4), (cs, w, d) in zip(ts_, ds):
                zd = sbp.tile([128, w], dt.float16, tag="zd")
                nc.vector.tensor_tensor(out=zd[:], in0=z[:], in1=d[:],
                                        op=OP.mult)
                zds.append((cs, zd))
            for cs, zd in zds:
                nc.vector.tensor_tensor(out=hT[:, cs], in0=hT[:, cs],
                                        in1=zd[:], op=OP.add)
                nc.sync.dma_start(out=out_h[:, cs], in_=hT[:, cs])

        def emit_gru_h(t, z, rh, agg4=None, tteng=None):
            """GRU phase 2 on tile t: candidate hc and the h update."""
            w = gtw(t)
            te = tteng or nc.vector
            cs = slice(512 * t, 512 * t + w)
            ph = gate(cs, 2, rh[:], agg4, w)
            hc = sbp.tile([128, w], dt.float16, tag="hc")
            nc.scalar.activation(hc[:], ph[:], ACT.Tanh, bias=bw[:, 2:3])
            d = sbp.tile([128, w], dt.float16, tag="d")
            nc.vector.tensor_tensor(out=d[:], in0=hc[:], in1=hT[:, cs],
                                    op=OP.subtract)
            zd = sbp.tile([128, w], dt.float16, tag="zd")
            te.tensor_tensor(out=zd[:], in0=z[:], in1=d[:],
                             op=OP.mult)
            te.tensor_tensor(out=hT[:, cs], in0=hT[:, cs], in1=zd[:],
                             op=OP.add)
            nc.sync.dma_start(out=out_h[:, cs], in_=hT[:, cs])

        if kind == "A":
            import os
            TUNE = dict(n_x=48, n_q8p=0, sbb=4, poff=2)
            for kv in os.environ.get("KTUNE", "").split(","):
                if ":" in kv:
                    k, v = kv.split(":")
                    TUNE[k] = int(v)
            NGRP = NM1 // 4
            n_x = (NGRP * TUNE["n_x"]) // 64
            n_q8p = (NGRP * TUNE["n_q8p"]) // 64
            POFF = TUNE["poff"]
            aggs = {}       # t -> agg4 tile (stage 1 emitted)
            zrh = {}        # t -> (z, rh) from GRU phase 1

            def bres(m, n, tot):
                return ((m + 1) * n) // tot > (m * n) // tot

            fired = set()

            def pump(step):
                tp, phase = divmod(step, 2)
                if tp < 0 or tp not in aggs or step in fired:
                    return
                fired.add(step)
                if phase == 0:
                    zrh[tp] = emit_gru_zr(tp, aggs[tp])
                else:
                    z, rh = zrh.pop(tp)
                    emit_gru_h(tp, z, rh, aggs.pop(tp))

            NBLK = -(-cfg["shard"] // 128)   # blocks with real atoms
            for t in range(NT):
                agg4 = agp.tile([128, 512, 2], dt.float16, tag="agg4")
                for half in range(BPT // 8):
                    m0 = t * BPT + half * 8
                    if m0 >= NBLK:
                        # all-padding half: aggregation is exactly zero
                        if half * 256 < gtw(t):
                            nc.gpsimd.memset(
                                agg4[:, half * 256:(half + 1) * 256, :], 0.0)
                        pump(2 * (t - POFF) + half)
                        continue
                    xb = sbp3.tile([36, 8, 512], dt.float16, tag="xb",
                                   bufs=TUNE["sbb"])
                    nc.sync.dma_start(
                        out=xb[:],
                        in_=xt4[m0:m0 + 8, :, :].rearrange("m k n -> k m n"))
                    for grp in range(2):
                        gidx = (t * (BPT // 8) + half) * 2 + grp
                        if m0 + grp * 4 >= NBLK:
                            if (half * 2 + grp) * 128 < gtw(t):
                                gsl = slice((half * 2 + grp) * 128,
                                            (half * 2 + grp + 1) * 128)
                                nc.gpsimd.memset(agg4[:, gsl, :], 0.0)
                            continue
                        rl4 = sbp3.tile([128, 4, 512], dt.float16, tag="rl4",
                                        bufs=TUNE["sbb"])
                        # evict pattern: X-groups relu blocks 2,3 on DVE,
                        # Y-groups do both pairs on ACT
                        is_x = bres(gidx, n_x, NGRP)
                        pma = ps1.tile([128, 1024], dt.float32,
                                       space="PSUM", tag="s1a", bufs=2)
                        for bi in range(2):
                            nc.tensor.matmul(
                                pma[:, bi * 512:(bi + 1) * 512], lhsT=wib[:],
                                rhs=xb[:, grp * 4 + bi, :],
                                start=True, stop=True)
                        nc.scalar.activation(
                            rl4[:, 0:2, :], pma[:], ACT.Relu)
                        if is_x:
                            for bi in (2, 3):
                                pmd = ps1.tile([128, 512], dt.float32,
                                               space="PSUM", tag="s1d",
                                               bufs=2)
                                nc.tensor.matmul(pmd[:], lhsT=wib[:],
                                                 rhs=xb[:, grp * 4 + bi, :],
                                                 start=True, stop=True)
                                nc.vector.tensor_scalar(
                                    out=rl4[:, bi, :], in0=pmd[:],
                                    scalar1=0.0, scalar2=None, op0=OP.max)
                        else:
                            pmb = ps1.tile([128, 1024], dt.float32,
                                           space="PSUM", tag="s1a", bufs=2)
                            for bi in range(2):
                                nc.tensor.matmul(
                                    pmb[:, bi * 512:(bi + 1) * 512],
                                    lhsT=wib[:],
                                    rhs=xb[:, grp * 4 + 2 + bi, :],
                                    start=True, stop=True)
                            nc.scalar.activation(
                                rl4[:, 2:4, :], pmb[:], ACT.Relu)
                        v = rl4[:].rearrange("p g (a k) -> p g a k", a=32)
                        q8 = sbp3.tile([128, 4, 32, 8], dt.float16, tag="q8",
                                       bufs=TUNE["sbb"])
                        q8eng = (nc.gpsimd if bres(gidx, n_q8p, NGRP)
                                 else nc.vector)
                        q8eng.tensor_tensor(
                            out=q8[:], in0=v[:, :, :, 0:8],
                            in1=v[:, :, :, 8:16], op=OP.add)
                        q4 = sbp3.tile([128, 4, 32, 4], dt.float16, tag="q4",
                                       bufs=TUNE["sbb"])
                        nc.gpsimd.tensor_tensor(
                            out=q4[:], in0=q8[:, :, :, 0:4],
                            in1=q8[:, :, :, 4:8], op=OP.add)
                        gsl = slice((half * 2 + grp) * 128,
                                    (half * 2 + grp + 1) * 128)
                        nc.gpsimd.tensor_tensor(
                            out=agg4[:, gsl, :].rearrange(
                                "p (g a) s -> p g a s", g=4),
                            in0=q4[:, :, :, 0:2], in1=q4[:, :, :, 2:4],
                            op=OP.add)
                    if t == 0 and half == 0:
                        emit_gw_dma(nc.scalar)
                        nc.scalar.dma_start(out=wia[:], in_=wia4[:])
                    pump(2 * (t - POFF) + half)
                # h0 for tile t
                csl = slice(512 * t, 512 * (t + 1))
                tfb = sbp.tile([32, 512], dt.float16, tag="tfb")
                nc.sync.dma_start(out=tfb[:], in_=tft4[:, csl])
                ph0 = ps1.tile([128, 512], dt.float32, space="PSUM",
                               tag="s1d", bufs=2)
                nc.tensor.matmul(ph0[:], lhsT=wia[:], rhs=tfb[:],
                                 start=True, stop=True)
                nc.scalar.copy(hT[:, csl], ph0[:])
                aggs[t] = agg4
            if NT >= 4:
                for step in range(2 * (NT - POFF), 2 * NT - 4):
                    pump(step)
                pump(2 * (NT - 2))      # zr(NT-2)
                pump(2 * (NT - 1))      # zr(NT-1)
                emit_gru_h_pair([
                    (NT - 2, *zrh.pop(NT - 2), aggs.pop(NT - 2)),
                    (NT - 1, *zrh.pop(NT - 1), aggs.pop(NT - 1))])
            else:
                for step in range(2 * (NT - POFF), 2 * NT):
                    pump(step)
        else:
            # 1024-wide tiles, two-phase software pipeline so PE never
            # convoys on the rh-dependent candidate matmul
            def bgate(t, wi, rhs2):
                w = widths[t]
                pm = psg.tile([128, w], dt.float32, space="PSUM", tag="g",
                              bufs=4)
                for h0 in range(0, w, 512):
                    hw_ = min(512, w - h0)
                    hs = slice(h0, h0 + hw_)
                    csh = slice(offs[t] + h0, offs[t] + h0 + hw_)
                    nc.tensor.matmul(pm[:, hs], lhsT=gw_s(2 * wi),
                                     rhs=agf[:, csh], start=True, stop=False)
                    nc.tensor.matmul(pm[:, hs], lhsT=gw_s(2 * wi + 1),
                                     rhs=rhs2[:, hs], start=False, stop=True)
                return pm

            creal = -(-cfg["shard"] // 4)    # cols with real atoms
            if COLS % 1024 == 0 and COLS // 1024 >= 4:
                n1024 = (COLS - 2048) // 1024
                widths = [512, 512] + [1024] * n1024 + [512, 512]
            else:
                widths = [512] * (COLS // 512)
            # trim trailing all-padding columns (host discards them)
            while sum(widths) - widths[-1] >= creal:
                widths.pop()
            tail = creal - (sum(widths) - widths[-1])
            widths[-1] = min(widths[-1], -(-tail // 8) * 8)
            offs = [sum(widths[:i]) for i in range(len(widths))]
            NTB = len(widths)
            for t in range(NTB):
                cs = slice(offs[t], offs[t] + widths[t])
                nc.sync.dma_start(out=agf[:, cs], in_=aggi[:, cs])
                nc.sync.dma_start(out=hT[:, cs], in_=hi[:, cs])

            def b_zr(t):
                cs = slice(offs[t], offs[t] + widths[t])
                pz = bgate(t, 0, hT[:, cs])
                z = sbp.tile([128, widths[t]], dt.float16, tag="z", bufs=4)
                nc.scalar.activation(z[:], pz[:], ACT.Sigmoid, bias=bw[:, 0:1])
                pr = bgate(t, 1, hT[:, cs])
                r = sbp.tile([128, widths[t]], dt.float16, tag="r")
                nc.scalar.activation(r[:], pr[:], ACT.Sigmoid, bias=bw[:, 1:2])
                rh = sbp.tile([128, widths[t]], dt.float16, tag="rh", bufs=4)
                nc.vector.tensor_tensor(out=rh[:], in0=r[:], in1=hT[:, cs],
                                        op=OP.mult)
                return z, rh

            def b_h(t, z, rh):
                cs = slice(offs[t], offs[t] + widths[t])
                ph = bgate(t, 2, rh[:])
                hc = sbp.tile([128, widths[t]], dt.float16, tag="hc")
                nc.scalar.activation(hc[:], ph[:], ACT.Tanh, bias=bw[:, 2:3])
                d = sbp.tile([128, widths[t]], dt.float16, tag="d")
                nc.vector.tensor_tensor(out=d[:], in0=hc[:], in1=hT[:, cs],
                                        op=OP.subtract)
                zd = sbp.tile([128, widths[t]], dt.float16, tag="zd")
                nc.vector.tensor_tensor(out=zd[:], in0=z[:], in1=d[:],
                                        op=OP.mult)
                nc.vector.tensor_tensor(out=hT[:, cs], in0=hT[:, cs],
                                        in1=zd[:], op=OP.add)
                nc.sync.dma_start(out=out_h[:, cs], in_=hT[:, cs])

            hist = {}
            for t in range(NTB):
                hist[t] = b_zr(t)
                if t >= 3:
                    b_h(t - 3, *hist.pop(t - 3))
            rem = sorted(hist)
            if len(rem) == 2:
                t1, t2 = rem
                z1, rh1 = hist.pop(t1)
                z2, rh2 = hist.pop(t2)
                ph1 = bgate(t1, 2, rh1[:])
                ph2 = bgate(t2, 2, rh2[:])
                cs1 = slice(offs[t1], offs[t1] + widths[t1])
                cs2 = slice(offs[t2], offs[t2] + widths[t2])
                hc1 = sbp.tile([128, widths[t1]], dt.float16, tag="hc")
                nc.scalar.activation(hc1[:], ph1[:], ACT.Tanh, bias=bw[:, 2:3])
                hc2 = sbp.tile([128, widths[t2]], dt.float16, tag="hc")
                nc.scalar.activation(hc2[:], ph2[:], ACT.Tanh, bias=bw[:, 2:3])
                d1 = sbp.tile([128, widths[t1]], dt.float16, tag="d")
                nc.vector.tensor_tensor(out=d1[:], in0=hc1[:], in1=hT[:, cs1],
                                        op=OP.subtract)
                d2 = sbp.tile([128, widths[t2]], dt.float16, tag="d")
                nc.vector.tensor_tensor(out=d2[:], in0=hc2[:], in1=hT[:, cs2],
                                        op=OP.subtract)
                zd1 = sbp.tile([128, widths[t1]], dt.float16, tag="zd")
                nc.vector.tensor_tensor(out=zd1[:], in0=z1[:], in1=d1[:],
                                        op=OP.mult)
                zd2 = sbp.tile([128, widths[t2]], dt.float16, tag="zd")
                nc.vector.tensor_tensor(out=zd2[:], in0=z2[:], in1=d2[:],
                                        op=OP.mult)
                nc.vector.tensor_tensor(out=hT[:, cs1], in0=hT[:, cs1],
                                        in1=zd1[:], op=OP.add)
                nc.sync.dma_start(out=out_h[:, cs1], in_=hT[:, cs1])
                nc.vector.tensor_tensor(out=hT[:, cs2], in0=hT[:, cs2],
                                        in1=zd2[:], op=OP.add)
                nc.sync.dma_start(out=out_h[:, cs2], in_=hT[:, cs2])
            else:
                for t in rem:
                    b_h(t, *hist.pop(t))

    nc.compile()
    _NC_CACHE[key] = nc
    return nc


def _pack4(x, cfg):
    """[SHARD_PAD, 32] row-major -> [128, COLS] 4-packed transposed."""
    return np.ascontiguousarray(
        x.reshape(cfg["cols"], 4, HID).transpose(1, 2, 0)).reshape(
        128, cfg["cols"])


def _unpack4(t4, cfg):
    return np.ascontiguousarray(
        t4.reshape(4, HID, cfg["cols"]).transpose(2, 0, 1)).reshape(-1, HID)


def kernel(**inputs):
    import os

    import ml_dtypes
    from concourse.bass_utils import run_bass_kernel_spmd as _run

    trace = bool(os.environ.get("KTRACE"))
    times = []

    def run_bass_kernel_spmd(nc, maps, core_ids):
        try:
            r = _run(nc, maps, core_ids=core_ids, trace=trace)
        except ModuleNotFoundError:
            # axon NTFF profiling hook unavailable in this image;
            # rerun without tracing rather than failing the launch
            r = _run(nc, maps, core_ids=core_ids, trace=False)
        if r.exec_time_ns:
            times.append(r.exec_time_ns)
        return r

    bf16 = np.float16

    tf = np.asarray(inputs["target_features"], np.float32)
    fdg = np.asarray(inputs["feature_dist_graph"], np.float32)
    rij = np.asarray(inputs["rij_dist_pairs"], np.float32)
    b_scope = np.asarray(inputs["b_scope"], np.int64)
    l_scope = np.asarray(inputs["l_scope"], np.int64)
    su = np.asarray(inputs["scope_update"], np.int64)
    sul = np.asarray(inputs["scope_update_lig"], np.int64)
    W_i_a = np.asarray(inputs["W_i_a"], np.float32)
    W_i_b = np.asarray(inputs["W_i_b"], np.float32)
    W_h = np.asarray(inputs["W_h"], np.float32)
    gW = {k: np.asarray(inputs["gru_W" + k], np.float32) for k in "zrh"}
    gb = {k: np.asarray(inputs["gru_b" + k], np.float32) for k in "zrh"}

    n_atoms = tf.shape[0]
    depth = gW["z"].shape[0]
    cfg = _cfg(n_atoms, depth)
    SHARD, SHARD_PAD, NM1 = cfg["shard"], cfg["shard_pad"], cfg["nm1"]

    valid = b_scope > 0
    pi = np.where(valid, b_scope - 1, 0)
    s1 = np.where(valid, su[pi], n_atoms)   # n_atoms -> zero row
    s2 = np.where(valid, sul[pi], n_atoms)
    ein = np.concatenate([fdg, rij[:, None]], axis=1)
    eidx_g = np.where(valid, pi, -1)

    def b4(w):
        return np.kron(np.eye(4, dtype=np.float32), w)

    def gru_weights(d):
        blocks = []
        for W in (gW["z"][d], gW["r"][d], gW["h"][d]):
            blocks.append(b4(W_h @ W[:HID]))
            blocks.append(b4(W[HID:]))
        gruw = np.ascontiguousarray(
            np.stack(blocks, axis=1).transpose(0, 1, 2)  # [128,6,128]
            .reshape(128, 6 * 128)).astype(bf16)
        biasw = np.stack([np.tile(gb[k][d], 4) for k in "zrh"],
                         axis=1).astype(np.float32)
        return gruw, biasw

    wia4 = b4(W_i_a).astype(bf16)
    wib4 = b4(W_i_b).astype(bf16)

    # ---- phase A inputs (stage 1 + h0 + GRU d=0) ----
    gruw0, biasw0 = gru_weights(0)
    in_maps = []
    for c in range(NCORES):
        lo = c * SHARD
        et = np.full((SHARD_PAD, 16), -1, np.int64)
        et[:SHARD] = eidx_g[lo:lo + SHARD]
        m_i = np.arange(NM1)[:, None, None, None]
        u_i = np.arange(4)[None, :, None, None]
        a_i = np.arange(32)[None, None, :, None]
        k_i = np.arange(16)[None, None, None, :]
        pid = et[4 * (32 * m_i + a_i) + u_i, k_i]
        feats = ein[np.clip(pid, 0, None)]
        feats[pid < 0] = 0.0
        xt4 = np.ascontiguousarray(feats.transpose(0, 1, 4, 2, 3)).reshape(
            NM1, 36, 512).astype(bf16)
        tfp = np.zeros((SHARD_PAD, FEAT), np.float32)
        tfp[:SHARD] = tf[lo:lo + SHARD]
        tft4 = np.ascontiguousarray(
            tfp.reshape(cfg["cols"], 4, FEAT).transpose(1, 2, 0)).reshape(
            32, cfg["cols"]).astype(bf16)
        in_maps.append(dict(xt4=xt4, tft4=tft4, gruw=gruw0, biasw=biasw0,
                            wia4=wia4, wib4=wib4))

    ncA = _build("A", cfg)
    res = run_bass_kernel_spmd(ncA, in_maps, core_ids=list(range(NCORES)))

    def collect_h(results):
        h = np.empty((n_atoms, HID), np.float32)
        for c in range(NCORES):
            h[c * SHARD:(c + 1) * SHARD] = _unpack4(
                np.asarray(results[c]["out_h"], np.float32), cfg)[:SHARD]
        return h

    def agg_prime(h):
        # sum of endpoint h rows over valid slots (static composed indices)
        hp = np.concatenate([h, np.zeros((1, HID), np.float32)], axis=0)
        return (hp[s1].sum(axis=1) + hp[s2].sum(axis=1)).astype(np.float32)

    h = collect_h(res.results)
    ncB = _build("B", cfg)
    for d in range(1, depth):
        ap = agg_prime(h)
        gruwd, biaswd = gru_weights(d)
        in_maps = []
        for c in range(NCORES):
            lo = c * SHARD
            apad = np.zeros((SHARD_PAD, HID), np.float32)
            apad[:SHARD] = ap[lo:lo + SHARD]
            hpad = np.zeros((SHARD_PAD, HID), np.float32)
            hpad[:SHARD] = h[lo:lo + SHARD]
            in_maps.append(dict(aggi=_pack4(apad, cfg).astype(bf16),
                                hi=_pack4(hpad, cfg).astype(bf16),
                                gruw=gruwd, biasw=biaswd))
        res = run_bass_kernel_spmd(ncB, in_maps, core_ids=list(range(NCORES)))
        h = collect_h(res.results)

    hp = np.concatenate([np.zeros((1, HID), np.float32), h], axis=0)
    if times:
        print("HW exec time: %d ns (sum of %d launches)"
              % (sum(times), len(times)))
    return hp[l_scope].sum(axis=1).astype(np.float32)
